# revision 10
# baseline (speedup 1.0000x reference)
"""MultiHeadGAT Trainium2 kernel (8 NeuronCores, data-parallel over batch).

Reference computation (per batch b of 32, n=512 nodes, d=128 feats, H=8 heads,
HID=64, top-k=16, leaky=0.2):
    h' = (h @ W).reshape(n, H, HID)                      # projection
    ei[g,i] = h'[i,g,:] . a_i[g];  ej[g,j] = h'[j,g,:] . a_j[g]
    e[g,i,j] = leaky_relu(ei[g,i] + ej[g,j])
    mask = topk_16(e, axis=j) | eye(n)
    attn = softmax(where(mask, e, -1e9))
    out = elu(attn @ h')

Key structural facts exploited:
  * leaky_relu is strictly monotone, and e[g,i,:] = leaky(ei[g,i] + ej[g,:]),
    so the top-16 column set J_g is THE SAME for every row i: it is the
    top-16 of the ej[g,:] vector. The attention matrix is therefore
    rank-17-structured: 16 shared columns + the diagonal.
  * softmax rows reduce to 17 candidates; computing only those is exact.
  * exp(leaky(x)) = max(exp(x), exp(0.2 x))  (exact identity).
  * elu(y) = max(y, exp(min(y,0)) - 1)  (exact identity).

v2 layout: attention weights are built TRANSPOSED (pexT[(g,c), i]) so the
output matmul needs no per-batch PE transpose / PSUM round-trip of the
softmax numerators. Denominators come out of block-ones matmuls re-using
pexT; the diagonal/top-k-threshold pipeline runs once globally on [32,512]
tiles (gpsimd + scalar), and normalization is pre-applied to pexT via a
PE broadcast of the reciprocals.

Per core: 4 batches. Sharding: batch across 8 cores, params replicated.
Host-side prep (untimed marshalling): h transposed to [b, d, n]; P = per-head
W @ a_{i,j} fold ([128,32] split-precision); output permuted on host.
"""
import sys
import numpy as np

sys.path.insert(0, "/opt/trn_rl_repo")

from contextlib import ExitStack

import concourse.bass as bass
import concourse.tile as tile
from concourse import bacc, mybir
from concourse.bass_utils import run_bass_kernel_spmd

f32 = mybir.dt.float32
bf16 = mybir.dt.bfloat16
AX = mybir.AxisListType
ALU = mybir.AluOpType
AF = mybir.ActivationFunctionType

N_HEADS = 8
HID = 64
TOP_K = 16
SLOPE = 0.2
BS, N, D = 32, 512, 128
CORES = 8
BPC = BS // CORES          # batches per core = 4
NCH = N // 128             # n-chunks = 4
GD = N_HEADS * HID         # 512


def _mid_bcast(ap, insert_at, counts_steps):
    """Insert [step, count] dims into an AP at position insert_at."""
    new = list(ap.ap)
    for step, count in reversed(counts_steps):
        new.insert(insert_at, [step, count])
    return bass.AP(ap.tensor, ap.offset, new)


def build_graph():
    nc = bacc.Bacc("TRN2", target_bir_lowering=False, debug=False)

    hTe_ext = nc.dram_tensor("hTe", [BPC, D, N], bf16, kind="ExternalInput")
    hTb_ext = nc.dram_tensor("hTb", [BPC, D, N], bf16, kind="ExternalInput")
    W_ext = nc.dram_tensor("W", [D, GD], bf16, kind="ExternalInput")
    P_ext = nc.dram_tensor("P", [D, 32], bf16, kind="ExternalInput")
    out_ext = nc.dram_tensor("out", [BPC, N, N_HEADS, HID], bf16,
                             kind="ExternalOutput")
    hT = hTe_ext.ap()
    hTb = hTb_ext.ap()
    Wap = W_ext.ap()
    Pap = P_ext.ap()
    outap = out_ext.ap()

    with tile.TileContext(nc) as tc, ExitStack() as ctx:
        const = ctx.enter_context(tc.tile_pool(name="const", bufs=1))
        sb = ctx.enter_context(tc.tile_pool(name="sb", bufs=3))
        sb4 = ctx.enter_context(tc.tile_pool(name="sb4", bufs=BPC))
        ps_hp = ctx.enter_context(
            tc.tile_pool(name="ps_hp", bufs=2, space="PSUM"))
        ps_o = ctx.enter_context(
            tc.tile_pool(name="ps_o", bufs=2, space="PSUM"))
        ps_hg = ctx.enter_context(
            tc.tile_pool(name="ps_hg", bufs=1, space="PSUM"))
        ps_mid = ctx.enter_context(
            tc.tile_pool(name="ps_mid", bufs=1, space="PSUM"))
        ps_small = ctx.enter_context(
            tc.tile_pool(name="ps_small", bufs=1, space="PSUM"))
        ps_den = ctx.enter_context(
            tc.tile_pool(name="ps_den", bufs=1, space="PSUM"))

        # ---------------- constants ----------------
        W_sb = const.tile([128, GD], bf16)
        nc.sync.dma_start(W_sb[:], Wap)
        P_sb = const.tile([128, 32], bf16)     # [Pb | Pe] split precision
        nc.sync.dma_start(P_sb[:], Pap)

        rowi = const.tile([128, 128], f32)
        nc.gpsimd.iota(rowi[:], [[1, 128]], channel_multiplier=0,
                       allow_small_or_imprecise_dtypes=True)
        coli = const.tile([128, 1], f32)
        nc.gpsimd.iota(coli[:], [[0, 1]], channel_multiplier=1,
                       allow_small_or_imprecise_dtypes=True)
        ident = const.tile([128, 128], f32)
        nc.vector.tensor_scalar(ident[:], rowi[:], coli[:], None,
                                op0=ALU.is_equal)

        identb = const.tile([128, 128], bf16)
        nc.vector.tensor_copy(identb[:], ident[:])

        ones32 = const.tile([32, 128], f32)
        nc.gpsimd.memset(ones32[:], 1.0)

        # block-diag mask: mblk[p, f] = (16*(f//64) <= p <= 16*(f//64)+15)
        colg_lo = const.tile([128, GD], f32)
        nc.gpsimd.iota(colg_lo[:].rearrange("p (g d) -> p g d", g=N_HEADS),
                       [[16, N_HEADS], [0, HID]], channel_multiplier=0,
                       allow_small_or_imprecise_dtypes=True)
        colg_hi = const.tile([128, GD], f32)
        nc.gpsimd.iota(colg_hi[:].rearrange("p (g d) -> p g d", g=N_HEADS),
                       [[16, N_HEADS], [0, HID]], base=15, channel_multiplier=0,
                       allow_small_or_imprecise_dtypes=True)
        mlo = const.tile([128, GD], f32)
        nc.vector.tensor_scalar(mlo[:], colg_lo[:], coli[:], None,
                                op0=ALU.is_le)
        mhi = const.tile([128, GD], f32)
        nc.vector.tensor_scalar(mhi[:], colg_hi[:], coli[:], None,
                                op0=ALU.is_ge)
        mblk = const.tile([128, GD], f32)
        nc.vector.tensor_tensor(mblk[:], mlo[:], mhi[:], op=ALU.mult)

        # E8f[k, (g,c)] = (k == g + 8), k in [0,16): lifts ei rows of the
        # score matrix to all 16 candidate partitions of head g (f32 matmul).
        e8v = const.tile([16, 128], f32)
        nc.gpsimd.iota(e8v[:].rearrange("p (g c) -> p g c", g=N_HEADS),
                       [[1, N_HEADS], [0, 16]], base=8, channel_multiplier=0,
                       allow_small_or_imprecise_dtypes=True)
        e8p = const.tile([16, 1], f32)
        nc.gpsimd.iota(e8p[:], [[0, 1]], channel_multiplier=1,
                       allow_small_or_imprecise_dtypes=True)
        E8f = const.tile([16, 128], f32)
        nc.vector.tensor_scalar(E8f[:], e8v[:], e8p[:], None, op0=ALU.is_equal)

        # JW[k, mm] = (k == mm//16) over mm in [0, 512): slicing
        # JW[:, 128b : 128b+128] gives (k == m//16 + 8b) for the recip bcast.
        jwv = const.tile([32, 512], f32)
        nc.gpsimd.iota(jwv[:].rearrange("p (q c) -> p q c", q=32),
                       [[1, 32], [0, 16]], channel_multiplier=0,
                       allow_small_or_imprecise_dtypes=True)
        jwp = const.tile([32, 1], f32)
        nc.gpsimd.iota(jwp[:], [[0, 1]], channel_multiplier=1,
                       allow_small_or_imprecise_dtypes=True)
        JW = const.tile([32, 512], f32)
        nc.vector.tensor_scalar(JW[:], jwv[:], jwp[:], None, op0=ALU.is_equal)

        # BW[p, mm] = (mm - 24 == p//16) over mm in [0,56); slicing
        # BW[:, 24-8b : 56-8b] gives the den-reduction lhsT for batch b.
        # Built like mblk: 16*(mm-24) <= p <= 16*(mm-24)+15
        bw_lo = const.tile([128, 64], f32)
        nc.gpsimd.iota(bw_lo[:], [[16, 64]], base=-24 * 16,
                       channel_multiplier=0,
                       allow_small_or_imprecise_dtypes=True)
        bw_hi = const.tile([128, 64], f32)
        nc.gpsimd.iota(bw_hi[:], [[16, 64]], base=-24 * 16 + 15,
                       channel_multiplier=0,
                       allow_small_or_imprecise_dtypes=True)
        bw_a = const.tile([128, 64], f32)
        nc.vector.tensor_scalar(bw_a[:], bw_lo[:], coli[:], None, op0=ALU.is_le)
        bw_b = const.tile([128, 64], f32)
        nc.vector.tensor_scalar(bw_b[:], bw_hi[:], coli[:], None, op0=ALU.is_ge)
        BWf = const.tile([128, 64], f32)
        nc.vector.tensor_tensor(BWf[:], bw_a[:], bw_b[:], op=ALU.mult)
        BW = const.tile([128, 64], bf16)
        nc.vector.tensor_copy(BW[:], BWf[:])

        # global score rows: T = ej[(b,g), n], Tei = ei[(b,g), n]
        T = const.tile([32, N], f32)
        Tei = const.tile([32, N], f32)

        # ---------------- stage A: projection + scores ----------------
        hT_sb = []
        hp_sb = []
        eij_sb = []
        ejt_sb = []
        for b in range(BPC):
            hte = sb.tile([128, N], bf16, tag="ht")
            nc.sync.dma_start(hte[:], hT[b])
            htb = sb.tile([128, N], bf16, tag="htb")
            nc.sync.dma_start(htb[:], hTb[b])
            hT_sb.append(hte)

            hp = sb4.tile([128, NCH, GD], bf16, tag="hp")
            for c in range(NCH):
                hp_ps = ps_hp.tile([128, GD], f32, tag="hp_ps")
                nc.tensor.matmul(hp_ps[:], htb[:, c * 128:(c + 1) * 128],
                                 W_sb[:], start=True, stop=True)
                if c % 2 == 0:
                    nc.scalar.copy(hp[:, c, :], hp_ps[:])
                else:
                    nc.vector.tensor_copy(hp[:, c, :], hp_ps[:])
            hp_sb.append(hp)

            # split-precision f32 score: ht@P = htb@Pb + htb@Pe + hte@Pb
            eijt_ps = ps_small.tile([16, N], f32, tag="small")
            nc.tensor.matmul(eijt_ps[:], P_sb[:, 0:16], htb[:],
                             start=True, stop=False)
            nc.tensor.matmul(eijt_ps[:], P_sb[:, 16:32], htb[:],
                             start=False, stop=False)
            nc.tensor.matmul(eijt_ps[:], P_sb[:, 0:16], hte[:],
                             start=False, stop=True)
            ejt16 = sb4.tile([16, N], f32, tag="ejt16")
            nc.scalar.copy(ejt16[:], eijt_ps[:])
            ejt_sb.append(ejt16)
            nc.sync.dma_start(T[b * 8:(b + 1) * 8, :], ejt16[0:8, :])
            nc.sync.dma_start(Tei[b * 8:(b + 1) * 8, :], ejt16[8:16, :])

            # ej in node-partition layout (for the one-hot gather compare)
            eij_ps = ps_small.tile([128, NCH, 8], f32, tag="small")
            for c in range(NCH):
                nc.tensor.transpose(eij_ps[:, c, :],
                                    ejt16[0:8, c * 128:(c + 1) * 128],
                                    ident[0:8, 0:8])
            eij = sb4.tile([128, NCH, 8], f32, tag="eij")
            nc.scalar.copy(eij[:], eij_ps[:])
            eij_sb.append(eij)

        # ---------------- stage B: top-16 of ej per (b,g) ----------------
        vals = const.tile([32, 16], f32)
        T2 = const.tile([32, N], f32)
        nc.vector.max(vals[:, 0:8], T[:])
        nc.vector.match_replace(T2[:], vals[:, 0:8], T[:], -1e30)
        nc.vector.max(vals[:, 8:16], T2[:])

        # ---------------- stage C1: pexT + gather, per batch ----------------
        pexT_sb = []
        hblk_sb = []
        den_ps = ps_den.tile([32, GD], f32, tag="den")
        for b in range(BPC):
            hp, eij = hp_sb[b], eij_sb[b]

            # rhsb[k, (g,c)] = vals[b*8+g, c] * (k == b*8+g)
            rhsb = sb.tile([32, N_HEADS, 16], f32, tag="rhsb")
            vals_mid = _mid_bcast(vals[:, 0:16], 1, [[0, N_HEADS]])
            id_sl = ident[0:32, b * 8:(b + 1) * 8].broadcast_to([32, 8, 16])
            nc.vector.tensor_tensor(rhsb[:], vals_mid, id_sl, op=ALU.mult)
            rhsb_f = rhsb[:].rearrange("k g c -> k (g c)")

            # vbc[i, (g,c)] = vals[g,c] (for the one-hot compare)
            vbc_ps = ps_small.tile([128, 128], f32, tag="small")
            nc.tensor.matmul(vbc_ps[:], ones32[:], rhsb_f,
                             start=True, stop=True)
            vbc = sb.tile([128, 128], f32, tag="vbc")
            nc.scalar.copy(vbc[:], vbc_ps[:])

            # vals128[(g,c), 0] = vals[g, c]  (per-partition bias vector)
            v128_ps = ps_small.tile([128, 1], f32, tag="small")
            nc.tensor.matmul(v128_ps[:], rhsb_f, ones32[:, 0:1],
                             start=True, stop=True)
            v128 = sb.tile([128, 2], f32, tag="v128")
            nc.vector.tensor_copy(v128[:, 0:1], v128_ps[:])
            nc.vector.tensor_scalar(v128[:, 1:2], v128_ps[:], SLOPE, None,
                                    op0=ALU.mult)

            # eibc[(g,c), i] = ei[g, i]  (f32 matmul broadcast; E8f's zero
            # rows null out the ej half, so contract over all 16 rows)
            eibc_ps = ps_mid.tile([128, N], f32, tag="mid")
            nc.tensor.matmul(eibc_ps[:], E8f[:], ejt_sb[b][:],
                             start=True, stop=True)

            # pexT = exp(leaky(vals + ei)) = max(exp(x), exp(0.2 x))
            ea = sb.tile([128, N], f32, tag="ea")
            nc.scalar.activation(ea[:], eibc_ps[:], AF.Exp,
                                 bias=v128[:, 0:1], scale=1.0)
            eb = sb.tile([128, N], f32, tag="eb")
            nc.scalar.activation(eb[:], eibc_ps[:], AF.Exp,
                                 bias=v128[:, 1:2], scale=SLOPE)
            pexT = sb4.tile([128, N], bf16, tag="pexT")
            nc.vector.tensor_tensor(pexT[:], ea[:], eb[:], op=ALU.max)
            pexT_sb.append(pexT)

            # den16[(b,g), i] += sum_c pexT[(g,c), i]
            nc.tensor.matmul(den_ps[:], BW[:, 24 - 8 * b:56 - 8 * b],
                             pexT[:], start=(b == 0), stop=(b == BPC - 1))

            # one-hot S[n, (g,c)] = (ej[n,g] == vals[g, c])  (gpsimd)
            S = sb4.tile([128, NCH, N_HEADS, 16], bf16, tag="S")
            ej_bc = eij[:].broadcast_to([128, NCH, N_HEADS, 16])
            vbc_rep = _mid_bcast(
                vbc[:].rearrange("p (g c) -> p g c", g=N_HEADS), 1, [[0, NCH]])
            nc.vector.tensor_tensor(S[:], ej_bc, vbc_rep, op=ALU.is_equal)

            # gathered rows H_gath[(g,c), :] = h'[j_gc, :], then block-mask
            hg_ps = ps_hg.tile([128, GD], f32, tag="hg_ps")
            for c in range(NCH):
                nc.tensor.matmul(hg_ps[:],
                                 S[:, c, :, :].rearrange("p g c -> p (g c)"),
                                 hp[:, c, :],
                                 start=(c == 0), stop=(c == NCH - 1))
            hblk = sb4.tile([128, GD], bf16, tag="hblk")
            nc.vector.tensor_tensor(hblk[:], hg_ps[:], mblk[:], op=ALU.mult)
            hblk_sb.append(hblk)

        # ---------------- global: diagonal + denominators ----------------
        sum32 = const.tile([32, N], f32)
        nc.vector.tensor_tensor(sum32[:], T[:], Tei[:], op=ALU.add)
        pa = const.tile([32, N], f32)
        nc.scalar.activation(pa[:], sum32[:], AF.Exp, scale=1.0)
        pb = const.tile([32, N], f32)
        nc.scalar.activation(pb[:], sum32[:], AF.Exp, scale=SLOPE)
        pd32 = const.tile([32, N], f32)
        nc.vector.tensor_tensor(pd32[:], pa[:], pb[:], op=ALU.max)
        # diagonal only counts when i not in J_g:  ej_i < t_g (16th largest)
        ind32 = const.tile([32, N], f32)
        nc.vector.tensor_scalar(ind32[:], T[:], vals[:, 15:16], None,
                                op0=ALU.is_lt)
        pdu32 = const.tile([32, N], f32)
        nc.gpsimd.tensor_tensor(pdu32[:], pd32[:], ind32[:], op=ALU.mult)
        dent32 = const.tile([32, N], f32)
        nc.vector.tensor_tensor(dent32[:], den_ps[:], pdu32[:], op=ALU.add)
        recip32 = const.tile([32, N], f32)
        nc.vector.reciprocal(recip32[:], dent32[:])
        pdn32 = const.tile([32, N], bf16)
        nc.gpsimd.tensor_tensor(pdn32[:], pdu32[:], recip32[:], op=ALU.mult)

        # pdn to node-partition layout: one transpose per chunk (all batches)
        pdnT_ps = ps_small.tile([128, NCH, 32], bf16, tag="small")
        for c in range(NCH):
            nc.tensor.transpose(pdnT_ps[:, c, :],
                                pdn32[:, c * 128:(c + 1) * 128],
                                identb[0:32, 0:32])
        pdnI = const.tile([128, NCH, 32], bf16)
        nc.scalar.copy(pdnI[:], pdnT_ps[:])

        # ---------------- stage C2: normalize + output ----------------
        for b in range(BPC):
            hp, pexT, hblk = hp_sb[b], pexT_sb[b], hblk_sb[b]

            # rbc[(g,c), i] = recip[b, g, i]
            rbc_ps = ps_mid.tile([128, N], f32, tag="mid")
            nc.tensor.matmul(rbc_ps[:], JW[:, 128 * b:128 * (b + 1)],
                             recip32[:], start=True, stop=True)
            qnT = sb4.tile([128, N], bf16, tag="qnT")
            nc.vector.tensor_tensor(qnT[:], pexT[:], rbc_ps[:], op=ALU.mult)

            # diagonal term dt = pdn * h' (gpsimd), injected via identity mm
            dt = sb.tile([128, NCH, N_HEADS, HID], bf16, tag="dt")
            for c in range(NCH):
                nc.gpsimd.tensor_tensor(
                    dt[:, c, :, :],
                    hp[:, c, :].rearrange("p (g d) -> p g d", g=N_HEADS),
                    pdnI[:, c, b * 8:(b + 1) * 8].broadcast_to(
                        [128, N_HEADS, HID]),
                    op=ALU.mult)

            for c in range(NCH):
                o_ps = ps_o.tile([128, GD], f32, tag="o_ps")
                nc.tensor.matmul(o_ps[:], qnT[:, c * 128:(c + 1) * 128],
                                 hblk[:], start=True, stop=False)
                nc.tensor.matmul(o_ps[:], identb[:],
                                 dt[:, c, :, :].rearrange("p g d -> p (g d)"),
                                 start=False, stop=True)
                # elu(y) = max(y, exp(min(y,0)) - 1)
                if c % 2 == 0:
                    rneg = sb.tile([128, GD], f32, tag="rneg")
                    nc.scalar.activation(rneg[:], o_ps[:], AF.Relu, scale=-1.0)
                else:
                    rneg = sb.tile([128, GD], f32, tag="rneg")
                    nc.vector.tensor_scalar(rneg[:], o_ps[:], 0.0, None,
                                            op0=ALU.min)
                sgn = -1.0 if c % 2 == 0 else 1.0
                vex = sb.tile([128, GD], f32, tag="vex")
                nc.scalar.activation(vex[:], rneg[:], AF.Exp, scale=sgn)
                ot = sb.tile([128, GD], bf16, tag="ot")
                nc.vector.scalar_tensor_tensor(ot[:], vex[:], 1.0, o_ps[:],
                                               op0=ALU.subtract, op1=ALU.max)
                nc.sync.dma_start(
                    outap[b, c * 128:(c + 1) * 128],
                    ot[:].rearrange("p (g d) -> p g d", g=N_HEADS))

    nc.compile()
    return nc


_CACHE = {}


def _get_graph():
    if "nc" not in _CACHE:
        _CACHE["nc"] = build_graph()
    return _CACHE["nc"]


def _prep_inputs(h, W, att_a):
    """Host-side marshalling: shard h over cores, transpose to [b,d,n],
    fold attention vectors into P = [W_g @ a_i_g | W_g @ a_j_g]."""
    h = np.asarray(h, dtype=np.float32)
    W = np.asarray(W, dtype=np.float32)
    att_a = np.asarray(att_a, dtype=np.float32)
    P = np.empty((D, 16), dtype=np.float32)
    for g in range(N_HEADS):
        Wg = W[:, g * HID:(g + 1) * HID]
        P[:, g] = Wg @ att_a[g, HID:]      # a_j -> ej (rows 0:8 of EIJT)
        P[:, 8 + g] = Wg @ att_a[g, :HID]  # a_i -> ei
    import ml_dtypes
    bfd = ml_dtypes.bfloat16
    Wb = W.astype(bfd)
    Pb = P.astype(bfd)
    Pe = (P - Pb.astype(np.float32)).astype(bfd)
    Pbe = np.ascontiguousarray(np.concatenate([Pb, Pe], axis=1))  # [D, 32]
    in_maps = []
    for core in range(CORES):
        hs = h[core * BPC:(core + 1) * BPC]            # [4, 512, 128]
        hTs = np.ascontiguousarray(hs.transpose(0, 2, 1))  # [4, 128, 512]
        hTb = hTs.astype(bfd)
        hTe = (hTs - hTb.astype(np.float32)).astype(bfd)
        in_maps.append({"hTe": hTe, "hTb": hTb, "W": Wb, "P": Pbe})
    return in_maps


def kernel(h, W, att_a):
    nc = _get_graph()
    in_maps = _prep_inputs(h, W, att_a)
    res = run_bass_kernel_spmd(nc, in_maps, list(range(CORES))).results
    outs = [r["out"].transpose(0, 2, 1, 3) for r in res]  # [4,H,n,d] each
    return np.ascontiguousarray(np.concatenate(outs, axis=0))
